# revision 5
# baseline (speedup 1.0000x reference)
"""Trainium2 Bass kernel for nn_Encoder_60455959659037 (VaDER-style encoder).

Data-parallel over batch across 8 NeuronCores (128 rows/core).
State in normal layout [b, h]; weights streamed as matmul rhs in fp32r
(full PE rate at N>=256); per-step activations transposed on the PE into
fp32r lhsT tiles. L0 runs reversed time; L1 consumes L0's hidden states
LIFO via a DRAM scratch buffer.
"""
import os
import numpy as np
import ml_dtypes

import concourse.bacc as bacc
import concourse.mybir as mybir
import concourse.tile as tile
from concourse.masks import make_identity
from concourse.bass_utils import run_bass_kernel_spmd

F32 = mybir.dt.float32
F32R = mybir.dt.float32r
BF16 = mybir.dt.bfloat16
AF = mybir.ActivationFunctionType
ALU = mybir.AluOpType

B, S_FULL, DTF, DS = 1024, 256, 64, 64
H, HS, DP = 512, 256, 64
NC_N = 8
BL = B // NC_N  # 128
HREP = HS + H  # 768
C2 = 2.7183

_BUILD_CACHE = {}


def _build(nsteps):
    nc = bacc.Bacc("TRN2", target_bir_lowering=False, debug=False)
    S = nsteps

    din = {}

    def ei(name, shape, dt):
        din[name] = nc.dram_tensor(name, list(shape), dt, kind="ExternalInput")

    ei("COVL", [BL, S_FULL * DTF], F32)
    ei("MCOVL", [BL, S_FULL * DTF], F32)
    ei("COVS", [BL, DS], F32)
    ei("MCOVS", [BL, DS], F32)
    ei("AT", [1, S_FULL * DTF], F32)
    ei("AS", [1, DS], F32)
    ei("Wx0x", [128, 4 * H], BF16)          # x-part of Wx0 duplicated in both partition halves, bf16
    ei("Wx0h", [4, 128, 4 * H], F32R)        # h-part of Wx0
    ei("Wd0", [4, 128, H], F32R)
    ei("Wx1", [8, 128, 4 * H], F32R)
    ei("Wd1", [4, 128, H], F32R)
    ei("Ws1", [DS, HS], F32R)
    ei("Ws2", [2, 128, HS], F32R)
    ei("Wm1", [6, 128, HREP], F32R)
    ei("Wm2", [6, 128, DP], F32R)
    ei("Wv1", [6, 128, HREP], F32R)
    ei("Wv2", [6, 128, DP], F32R)

    rep_o = nc.dram_tensor("rep", [BL, HREP], F32, kind="ExternalOutput")
    mean_o = nc.dram_tensor("mean", [BL, DP], F32, kind="ExternalOutput")
    var_o = nc.dram_tensor("var", [BL, DP], F32, kind="ExternalOutput")

    with tile.TileContext(nc) as tc:
        _emit(nc, tc, din, rep_o, mean_o, var_o, S)
    nc.compile()
    return nc


def _emit(nc, tc, din, rep_o, mean_o, var_o, S):
    N_ORDER = [0, 3, 1, 2]  # gate chunk order: i, g, f, o

    with tc.tile_pool(name="persist", bufs=1) as persist, \
         tc.tile_pool(name="state", bufs=2) as state, \
         tc.tile_pool(name="stateT", bufs=2) as stateT, \
         tc.tile_pool(name="h0in", bufs=4) as h0in, \
         tc.tile_pool(name="dram", bufs=1, space="DRAM") as dramp, \
         tc.tile_pool(name="ps_big", bufs=1, space="PSUM") as ps_big, \
         tc.tile_pool(name="ps_cst", bufs=2, space="PSUM") as ps_cst, \
         tc.tile_pool(name="ps_tr", bufs=2, space="PSUM") as ps_tr:

        ident = persist.tile([128, 128], F32)
        make_identity(nc, ident)
        Tm1 = persist.tile([128, S], F32)
        dt_all = persist.tile([128, S], F32)
        rep_stat = persist.tile([128, HS], F32)
        Wd1_sb = persist.tile([128, 4, H], F32R)
        nc.sync.dma_start(out=Wd1_sb, in_=din["Wd1"].ap().transpose([1, 0, 2]))

        h0T_dram = dramp.tile([S, 128, H], F32R)

        # ---------------- scan step ----------------
        def make_state():
            h = state.tile([128, H], F32, tag="h")
            c = state.tile([128, H], F32, tag="c")
            hT = stateT.tile([128, 4, 128], F32R, tag="hT")
            nc.vector.memset(h, 0.0)
            nc.gpsimd.memset(c, 0.0)
            nc.vector.memset(hT.bitcast(F32), 0.0)
            return h, c, hT

        def trans4(src, dst, tag="tr"):
            """PE-transpose four [128,128] blocks of src into dst [128,4,128]."""
            for k in range(4):
                tp = ps_tr.tile([128, 128], F32, tag=tag)
                nc.tensor.transpose(tp, src[:, k * 128:(k + 1) * 128], ident)
                if k % 2 == 0:
                    nc.scalar.activation(dst[:, k, :], tp, AF.Copy)
                else:
                    nc.vector.tensor_copy(dst[:, k, :], tp)

        def step(layer, j, h, c, hT, Wd_sb, gates_mm, h0T_tile):
            o = S - 1 - j
            # cst = tanh(h @ Wd)
            cst_ps = ps_cst.tile([128, H], F32, tag="cst")
            for k in range(4):
                nc.tensor.matmul(cst_ps, hT[:, k, :], Wd_sb[:, k, :],
                                 start=(k == 0), stop=(k == 3))
            cst = state.tile([128, H], F32, tag="cst_t")
            nc.scalar.activation(cst, cst_ps, AF.Tanh)
            # h_adj = h + Tm1[o] * cst
            h_adj = state.tile([128, H], F32, tag="h_adj")
            nc.vector.scalar_tensor_tensor(
                out=h_adj, in0=cst, scalar=Tm1[:, o:o + 1], in1=h,
                op0=ALU.mult, op1=ALU.add)
            h_adjT = stateT.tile([128, 4, 128], F32R, tag="h_adjT")
            trans4(h_adj, h_adjT)
            # gates
            g_ps = ps_big.tile([128, 4 * H], F32, tag="g")
            gates_mm(g_ps, o, h_adjT, h0T_tile)
            sig_i = state.tile([128, H], F32, tag="sig_i")
            nc.scalar.activation(sig_i, g_ps[:, 0:H], AF.Sigmoid)
            tanh_g = state.tile([128, H], F32, tag="tanh_g")
            nc.scalar.activation(tanh_g, g_ps[:, 3 * H:4 * H], AF.Tanh)
            t1 = state.tile([128, H], F32, tag="t1")
            nc.vector.tensor_mul(t1, sig_i, tanh_g)
            sig_f = state.tile([128, H], F32, tag="sig_f")
            nc.scalar.activation(sig_f, g_ps[:, H:2 * H], AF.Sigmoid)
            t2 = state.tile([128, H], F32, tag="t2")
            nc.gpsimd.tensor_mul(t2, sig_f, c)
            sig_o = state.tile([128, H], F32, tag="sig_o")
            nc.scalar.activation(sig_o, g_ps[:, 2 * H:3 * H], AF.Sigmoid)
            c_n = state.tile([128, H], F32, tag="c")
            nc.vector.tensor_add(c_n, t1, t2)
            tanh_c = state.tile([128, H], F32, tag="tanh_c")
            nc.scalar.activation(tanh_c, c_n, AF.Tanh)
            h_n = state.tile([128, H], F32, tag="h")
            nc.vector.tensor_mul(h_n, sig_o, tanh_c)
            hT_n = stateT.tile([128, 4, 128], F32R, tag="hT")
            trans4(h_n, hT_n)
            return h_n, c_n, hT_n

        # ---------------- L0 (+ pre-phase) ----------------
        with tc.tile_pool(name="l0w", bufs=1) as l0w, \
             tc.tile_pool(name="covlTp", bufs=1) as covlT_pool:
            Wx0x_sb = l0w.tile([128, 4 * H], BF16)
            nc.sync.dma_start(out=Wx0x_sb, in_=din["Wx0x"].ap())
            Wx0h_sb = l0w.tile([128, 4, 4 * H], F32R)
            nc.sync.dma_start(out=Wx0h_sb, in_=din["Wx0h"].ap().transpose([1, 0, 2]))
            Wd0_sb = l0w.tile([128, 4, H], F32R)
            nc.sync.dma_start(out=Wd0_sb, in_=din["Wd0"].ap().transpose([1, 0, 2]))

            # covlT[:, p, :]: rows 0:64 = orig step 2p, rows 64:128 = step 2p+1 (transposed)
            covlT = covlT_pool.tile([128, S // 2, 128], BF16)

            with tc.tile_pool(name="pre", bufs=2) as pre:
                CH = 16
                for ci in range((S + CH - 1) // CH):
                    s0 = ci * CH
                    ns = min(CH, S - s0)
                    w = ns * DTF
                    cC = pre.tile([128, CH * DTF], F32, tag="cC")
                    cM = pre.tile([128, CH * DTF], F32, tag="cM")
                    cA = pre.tile([128, CH * DTF], F32, tag="cA")
                    nc.sync.dma_start(out=cC[:, :w], in_=din["COVL"].ap()[:, s0 * DTF:(s0 + ns) * DTF])
                    nc.sync.dma_start(out=cM[:, :w], in_=din["MCOVL"].ap()[:, s0 * DTF:(s0 + ns) * DTF])
                    nc.sync.dma_start(
                        out=cA[:, :w],
                        in_=din["AT"].ap()[:, s0 * DTF:(s0 + ns) * DTF].partition_broadcast(128).squeeze(1))
                    cv = pre.tile([128, CH * DTF], F32, tag="cv")
                    nc.vector.tensor_sub(cv[:, :w], cC[:, :w], cA[:, :w])
                    nc.vector.tensor_mul(cv[:, :w], cM[:, :w], cv[:, :w])
                    nc.vector.tensor_add(cv[:, :w], cA[:, :w], cv[:, :w])
                    cvv = cv.rearrange("p (s d) -> p s d", d=DTF)
                    nc.gpsimd.tensor_copy(dt_all[:, s0:s0 + ns], cvv[:, 0:ns, DTF - 1])
                    for bi in range(ns // 2):
                        tp = ps_tr.tile([128, 128], F32, tag="tr")
                        nc.tensor.transpose(tp, cv[:, bi * 128:(bi + 1) * 128], ident)
                        pi = s0 // 2 + bi
                        if pi % 2 == 0:
                            nc.scalar.activation(covlT[:, pi, :], tp, AF.Copy)
                        else:
                            nc.vector.tensor_copy(covlT[:, pi, :], tp)

                # Tm1 = 1/ln(dt + C2) - 1
                c2t = pre.tile([128, 1], F32, tag="c2t")
                nc.vector.memset(c2t, float(C2))
                lnv = pre.tile([128, S], F32, tag="lnv")
                nc.scalar.activation(lnv, dt_all, AF.Ln, bias=c2t)
                nc.vector.reciprocal(Tm1, lnv)
                nc.vector.tensor_scalar_add(Tm1, Tm1, -1.0)

                # static encoder
                sC = pre.tile([128, DS], F32, tag="sC")
                sM = pre.tile([128, DS], F32, tag="sM")
                sA = pre.tile([128, DS], F32, tag="sA")
                nc.sync.dma_start(out=sC, in_=din["COVS"].ap())
                nc.sync.dma_start(out=sM, in_=din["MCOVS"].ap())
                nc.sync.dma_start(out=sA, in_=din["AS"].ap().partition_broadcast(128).squeeze(1))
                sv = pre.tile([128, DS], F32, tag="sv")
                nc.vector.tensor_sub(sv, sC, sA)
                nc.vector.tensor_mul(sv, sM, sv)
                nc.vector.tensor_add(sv, sA, sv)
                tp = ps_tr.tile([128, 128], F32, tag="tr")
                nc.tensor.transpose(tp[0:DS, :], sv, ident)
                covsT = pre.tile([DS, 128], F32R, tag="covsT")
                nc.vector.tensor_copy(covsT, tp[0:DS, :])
                Ws1_sb = pre.tile([DS, HS], F32R, tag="Ws1")
                nc.sync.dma_start(out=Ws1_sb, in_=din["Ws1"].ap())
                Ws2_sb = pre.tile([128, 2, HS], F32R, tag="Ws2")
                nc.sync.dma_start(out=Ws2_sb, in_=din["Ws2"].ap().transpose([1, 0, 2]))
                st1_ps = ps_cst.tile([128, H], F32, tag="cst")
                nc.tensor.matmul(st1_ps[:, 0:HS], covsT, Ws1_sb, start=True, stop=True)
                st1 = pre.tile([128, HS], F32, tag="st1")
                nc.scalar.activation(st1, st1_ps[:, 0:HS], AF.Relu)
                st1T = pre.tile([128, 2, 128], F32R, tag="st1T")
                for k in range(2):
                    tp = ps_tr.tile([128, 128], F32, tag="tr")
                    nc.tensor.transpose(tp, st1[:, k * 128:(k + 1) * 128], ident)
                    nc.vector.tensor_copy(st1T[:, k, :], tp)
                st2_ps = ps_cst.tile([128, H], F32, tag="cst")
                for k in range(2):
                    nc.tensor.matmul(st2_ps[:, 0:HS], st1T[:, k, :], Ws2_sb[:, k, :],
                                     start=(k == 0), stop=(k == 1))
                nc.scalar.activation(rep_stat, st2_ps[:, 0:HS], AF.Copy)
                nc.sync.dma_start(out=rep_o.ap()[:, 0:HS], in_=rep_stat)

            # L0 gates
            def gates_l0(g_ps, o, h_adjT, _h0T):
                p, sub = o // 2, o % 2
                xT = covlT[sub * 64:(sub + 1) * 64, p, :]
                for n in N_ORDER:
                    gn = g_ps[:, n * H:(n + 1) * H]
                    nc.tensor.matmul(gn, xT,
                                     Wx0x_sb[sub * 64:(sub + 1) * 64, n * H:(n + 1) * H],
                                     start=True, stop=False)
                    for k in range(4):
                        nc.tensor.matmul(gn, h_adjT[:, k, :],
                                         Wx0h_sb[:, k, n * H:(n + 1) * H],
                                         start=False, stop=(k == 3))

            h, c, hT = make_state()
            for j in range(S):
                h, c, hT = step(0, j, h, c, hT, Wd0_sb, gates_l0, None)
                nc.sync.dma_start(out=h0T_dram[j], in_=hT)

        # ---------------- L1 + heads ----------------
        with tc.tile_pool(name="l1w", bufs=1) as l1w:
            Wx1_sb = l1w.tile([128, 8, 4 * H], F32R)
            for k2 in range(4):
                nc.sync.dma_start(
                    out=Wx1_sb[:, 2 * k2:2 * k2 + 2, :],
                    in_=din["Wx1"].ap()[2 * k2:2 * k2 + 2].transpose([1, 0, 2]))
            Wm1_sb = l1w.tile([128, 6, HREP], F32R)
            nc.sync.dma_start(out=Wm1_sb, in_=din["Wm1"].ap().transpose([1, 0, 2]))
            Wm2_sb = l1w.tile([128, 6, DP], F32R)
            nc.sync.dma_start(out=Wm2_sb, in_=din["Wm2"].ap().transpose([1, 0, 2]))
            Wv1_sb = l1w.tile([128, 6, HREP], F32R)
            nc.sync.dma_start(out=Wv1_sb, in_=din["Wv1"].ap().transpose([1, 0, 2]))
            Wv2_sb = l1w.tile([128, 6, DP], F32R)
            nc.sync.dma_start(out=Wv2_sb, in_=din["Wv2"].ap().transpose([1, 0, 2]))

            def gates_l1(g_ps, o, h_adjT, h0T_tile):
                h0v = h0T_tile.rearrange("p (k b) -> p k b", k=4)
                for n in N_ORDER:
                    gn = g_ps[:, n * H:(n + 1) * H]
                    for k in range(8):
                        lhs = h0v[:, k, :] if k < 4 else h_adjT[:, k - 4, :]
                        nc.tensor.matmul(gn, lhs, Wx1_sb[:, k, n * H:(n + 1) * H],
                                         start=(k == 0), stop=(k == 7))

            PF = 3  # prefetch depth (h0in bufs=4)

            def fetch_h0(j):
                t = h0in.tile([128, H], F32R, tag="h0")
                nc.sync.dma_start(out=t, in_=h0T_dram[S - 1 - j])
                return t

            h0_tiles = {}
            for j in range(min(PF, S)):
                h0_tiles[j] = fetch_h0(j)
            h, c, hT = make_state()
            for j in range(S):
                if j + PF < S:
                    h0_tiles[j + PF] = fetch_h0(j + PF)
                h, c, hT = step(1, j, h, c, hT, Wd1_sb, gates_l1, h0_tiles.pop(j))

            # rep[:, HS:] = final h
            nc.sync.dma_start(out=rep_o.ap()[:, HS:HREP], in_=h)

            # heads: mean = rep @ Wm1 @ Wm2 ; var analogous
            statT = stateT.tile([128, 2, 128], F32R, tag="statT")
            for k in range(2):
                tp = ps_tr.tile([128, 128], F32, tag="tr")
                nc.tensor.transpose(tp, rep_stat[:, k * 128:(k + 1) * 128], ident)
                nc.vector.tensor_copy(statT[:, k, :], tp)

            def rep_chunk(k):
                return statT[:, k, :] if k < 2 else hT[:, k - 2, :]

            def head(W1_sb, W2_sb, out_dram):
                h1_ps = ps_big.tile([128, 4 * H], F32, tag="g")
                for n, (n0, n1) in enumerate([(0, H), (H, HREP)]):
                    for k in range(6):
                        nc.tensor.matmul(h1_ps[:, n0:n1], rep_chunk(k),
                                         W1_sb[:, k, n0:n1],
                                         start=(k == 0), stop=(k == 5))
                h1 = state.tile([128, HREP], F32, tag="head1")
                nc.scalar.activation(h1, h1_ps[:, 0:HREP], AF.Copy)
                h1T = stateT.tile([128, 6, 128], F32R, tag="headT")
                for k in range(6):
                    tp = ps_tr.tile([128, 128], F32, tag="tr")
                    nc.tensor.transpose(tp, h1[:, k * 128:(k + 1) * 128], ident)
                    if k % 2 == 0:
                        nc.scalar.activation(h1T[:, k, :], tp, AF.Copy)
                    else:
                        nc.vector.tensor_copy(h1T[:, k, :], tp)
                h2_ps = ps_cst.tile([128, H], F32, tag="cst")
                for k in range(6):
                    nc.tensor.matmul(h2_ps[:, 0:DP], h1T[:, k, :], W2_sb[:, k, :],
                                     start=(k == 0), stop=(k == 5))
                h2 = state.tile([128, DP], F32, tag="head2")
                nc.scalar.activation(h2, h2_ps[:, 0:DP], AF.Copy)
                nc.sync.dma_start(out=out_dram.ap()[:, :], in_=h2)

            head(Wm1_sb, Wm2_sb, mean_o)
            head(Wv1_sb, Wv2_sb, var_o)


# ---------------- host side ----------------

def _dup_wx0x(Wx0):
    bf = ml_dtypes.bfloat16
    d = np.zeros((128, 4 * H), np.float32)
    d[0:DTF - 1] = Wx0[:DTF - 1]
    d[64:64 + DTF - 1] = Wx0[:DTF - 1]
    return d.astype(bf)


def _prep_weights(inp):
    bf = ml_dtypes.bfloat16
    w = {
        "Wx0x": _dup_wx0x(inp["Wx0"]),
        "Wx0h": np.ascontiguousarray(inp["Wx0"][DTF - 1:].reshape(4, 128, 4 * H)),
        "Wd0": np.ascontiguousarray(inp["Wd0"].reshape(4, 128, H)),
        "Wx1": np.ascontiguousarray(inp["Wx1"].reshape(8, 128, 4 * H)),
        "Wd1": np.ascontiguousarray(inp["Wd1"].reshape(4, 128, H)),
        "Ws1": np.ascontiguousarray(inp["Ws1"]),
        "Ws2": np.ascontiguousarray(inp["Ws2"].reshape(2, 128, HS)),
        "Wm1": np.ascontiguousarray(inp["Wm1"].reshape(6, 128, HREP)),
        "Wm2": np.ascontiguousarray(inp["Wm2"].reshape(6, 128, DP)),
        "Wv1": np.ascontiguousarray(inp["Wv1"].reshape(6, 128, HREP)),
        "Wv2": np.ascontiguousarray(inp["Wv2"].reshape(6, 128, DP)),
        "AT": np.ascontiguousarray(inp["AT"].reshape(1, S_FULL * DTF)),
        "AS": np.ascontiguousarray(inp["AS"].reshape(1, DS)),
    }
    return w


def kernel(**inputs):
    inp = {k: np.asarray(v, dtype=np.float32) if np.asarray(v).dtype == np.float32
           else np.asarray(v) for k, v in inputs.items()}
    for bn in ("bx0", "bd0", "bx1", "bd1", "bs1", "bs2", "bm1", "bm2", "bv1", "bv2"):
        assert not np.any(inp[bn]), f"nonzero bias {bn} not supported by this kernel build"

    nsteps = int(os.environ.get("KERNEL_STEPS", S_FULL))
    if nsteps not in _BUILD_CACHE:
        _BUILD_CACHE[nsteps] = _build(nsteps)
    nc = _BUILD_CACHE[nsteps]

    w = _prep_weights(inp)
    in_maps = []
    for i in range(NC_N):
        sl = slice(i * BL, (i + 1) * BL)
        m = dict(w)
        m["COVL"] = np.ascontiguousarray(inp["COVL"][sl].reshape(BL, S_FULL * DTF))
        m["MCOVL"] = np.ascontiguousarray(inp["MCOVL"][sl].reshape(BL, S_FULL * DTF))
        m["COVS"] = np.ascontiguousarray(inp["COVS"][sl])
        m["MCOVS"] = np.ascontiguousarray(inp["MCOVS"][sl])
        in_maps.append(m)

    res = run_bass_kernel_spmd(nc, in_maps, core_ids=list(range(NC_N)))
    rep = np.concatenate([res.results[i]["rep"] for i in range(NC_N)], axis=0)
    mean = np.concatenate([res.results[i]["mean"] for i in range(NC_N)], axis=0)
    var = np.concatenate([res.results[i]["var"] for i in range(NC_N)], axis=0)
    return rep, mean, var
